# revision 1
# baseline (speedup 1.0000x reference)
"""Trainium2 Bass kernel for CustomQuantizedLinear.

Computes out[b,s,o] = sum_i x[b,s,i] * ((q[o,i]-128)*0.02) + bias[o]
for x (4,2048,4096) f32, q (4096,4096) int32, bias (4096,) f32.

Sharding across 8 NeuronCores: 4 token groups x 2 out-feature groups.
Each core computes a (2048 tokens, 2048 out-features) block of the
flattened (8192, 4096) output.

Per-core dataflow (all matmul compute in bf16, accumulate f32):
  - weights: DMA int32 -> dequant to bf16 on ScalarE (one activation op:
    Copy(q*0.02 - 2.56)) -> PE-transpose 128x128 blocks -> resident
    [K, out] bf16 tiles in SBUF (16 MB).
  - x: DMA f32 -> cast to bf16 (ScalarE) -> PE-transpose -> [K, tok]
    bf16 stationary tiles.
  - matmul: out[tok, o] += xT.T @ wT over 32 k-tiles into PSUM.
  - eviction: VectorE adds broadcast bias while copying PSUM->SBUF,
    then DMA to DRAM.
"""

import numpy as np

SCALE = 0.02
ZERO_POINT = 128

B, S, K, O = 4, 2048, 4096, 4096
N_CORES = 8
TOK_GROUPS, OUT_GROUPS = 4, 2
TOK_PC = B * S // TOK_GROUPS  # 2048 tokens per core
OUT_PC = O // OUT_GROUPS      # 2048 out features per core

_BUILD_CACHE = {}


def _build_bass(tok_pc=TOK_PC, out_pc=OUT_PC, k=K):
    """Build + compile the per-core Bass program. Returns (nc, names)."""
    from contextlib import ExitStack

    import concourse.mybir as mybir
    import concourse.tile as tile
    from concourse import bacc
    from concourse.masks import make_identity

    f32 = mybir.dt.float32
    bf16 = mybir.dt.bfloat16
    i32 = mybir.dt.int32
    ADD = mybir.AluOpType.add
    Copy = mybir.ActivationFunctionType.Copy

    P = 128
    FREE = 512                 # matmul moving free dim (one PSUM bank of f32)
    KT = k // P                # number of k tiles
    TOKT = tok_pc // P         # number of token tiles
    OC = out_pc // FREE        # out chunks of 512
    OTILES = out_pc // P       # out row tiles (128 o each)
    OT_PER_OC = FREE // P      # row tiles per out chunk
    HALF = min(k, 2048)        # staging width per DMA
    NH = k // HALF
    KL = HALF // P

    nc = bacc.Bacc(None, target_bir_lowering=False)
    with tile.TileContext(nc) as tc:
        with ExitStack() as ctx:
            dram = ctx.enter_context(tc.tile_pool(name="dram", bufs=1, space="DRAM"))
            x_d = dram.tile([tok_pc, k], f32, kind="ExternalInput", name="x_in")
            w_d = dram.tile([out_pc, k], i32, kind="ExternalInput", name="w_in")
            b_d = dram.tile([1, out_pc], f32, kind="ExternalInput", name="b_in")
            o_d = dram.tile([tok_pc, out_pc], f32, kind="ExternalOutput", name="o_out")

            const = ctx.enter_context(tc.tile_pool(name="const", bufs=1))
            stage = ctx.enter_context(tc.tile_pool(name="stage", bufs=3))
            bfst = ctx.enter_context(tc.tile_pool(name="bfst", bufs=3))
            wtp = ctx.enter_context(tc.tile_pool(name="wtp", bufs=1))
            xtp = ctx.enter_context(tc.tile_pool(name="xtp", bufs=2))
            outp = ctx.enter_context(tc.tile_pool(name="outp", bufs=4))
            pst = ctx.enter_context(tc.tile_pool(name="pst", bufs=3, space="PSUM"))
            psm = ctx.enter_context(tc.tile_pool(name="psm", bufs=4, space="PSUM"))

            ident = const.tile([P, P], bf16, name="ident")
            make_identity(nc, ident)
            ones_k = const.tile([1, P], f32, name="ones_k")
            nc.gpsimd.memset(ones_k, 1.0)

            # bias: load [1, out_pc], broadcast to 128 partitions via a
            # K=1 matmul with a ones stationary.
            bias_sb = const.tile([1, out_pc], f32, name="bias_sb")
            nc.sync.dma_start(bias_sb, b_d[:, :])
            bias_rep = const.tile([P, out_pc], f32, name="bias_rep")
            for oc in range(OC):
                pb = psm.tile([P, FREE], f32, tag="acc", name=f"pb{oc}")
                nc.tensor.matmul(
                    pb, lhsT=ones_k, rhs=bias_sb[:, oc * FREE:(oc + 1) * FREE],
                    start=True, stop=True,
                )
                nc.scalar.copy(bias_rep[:, oc * FREE:(oc + 1) * FREE], pb)

            # resident transposed dequantized weights: per out-chunk
            # [128 k-part, KT, 512 o] bf16
            wt = [wtp.tile([P, KT, FREE], bf16, name=f"wt{oc}") for oc in range(OC)]

            def prep_w(ot):
                oc, col = divmod(ot, OT_PER_OC)
                for h in range(NH):
                    wstage = stage.tile([P, HALF], i32, tag="stage",
                                        name=f"wst_{ot}_{h}")
                    nc.sync.dma_start(
                        wstage, w_d[ot * P:(ot + 1) * P, h * HALF:(h + 1) * HALF])
                    wbf = bfst.tile([P, HALF], bf16, tag="bfst", name=f"wbf_{ot}_{h}")
                    nc.scalar.activation(
                        wbf, wstage, Copy,
                        bias=float(-ZERO_POINT * SCALE), scale=float(SCALE))
                    for kl in range(KL):
                        ki = h * KL + kl
                        pt = pst.tile([P, P], bf16, tag="pt", name=f"wpt_{ot}_{h}_{kl}")
                        nc.tensor.transpose(pt, wbf[:, kl * P:(kl + 1) * P], ident)
                        dst = wt[oc][:, ki, col * P:(col + 1) * P]
                        if ki % 2 == 0:
                            nc.vector.tensor_copy(dst, pt)
                        else:
                            nc.scalar.copy(dst, pt)

            def do_tok(tt):
                xt = xtp.tile([P, KT, P], bf16, tag="xt", name=f"xt{tt}")
                for h in range(NH):
                    xstage = stage.tile([P, HALF], f32, tag="stage",
                                        name=f"xst_{tt}_{h}")
                    nc.sync.dma_start(
                        xstage, x_d[tt * P:(tt + 1) * P, h * HALF:(h + 1) * HALF])
                    xbf = bfst.tile([P, HALF], bf16, tag="bfst", name=f"xbf_{tt}_{h}")
                    nc.scalar.activation(xbf, xstage, Copy)
                    for kl in range(KL):
                        ki = h * KL + kl
                        pt = pst.tile([P, P], bf16, tag="pt", name=f"xpt_{tt}_{h}_{kl}")
                        nc.tensor.transpose(pt, xbf[:, kl * P:(kl + 1) * P], ident)
                        dst = xt[:, ki, :]
                        if ki % 2 == 0:
                            nc.scalar.copy(dst, pt)
                        else:
                            nc.vector.tensor_copy(dst, pt)
                for oc in range(OC):
                    acc = psm.tile([P, FREE], f32, tag="acc", name=f"acc_{tt}_{oc}")
                    for ki in range(KT):
                        nc.tensor.matmul(
                            acc, lhsT=xt[:, ki, :], rhs=wt[oc][:, ki, :],
                            start=(ki == 0), stop=(ki == KT - 1))
                    ot_sb = outp.tile([P, FREE], f32, tag="outt", name=f"o_{tt}_{oc}")
                    nc.vector.tensor_tensor(
                        ot_sb, acc, bias_rep[:, oc * FREE:(oc + 1) * FREE], ADD)
                    nc.sync.dma_start(
                        o_d[tt * P:(tt + 1) * P, oc * FREE:(oc + 1) * FREE], ot_sb)

            for ot in range(OTILES):
                prep_w(ot)
            for tt in range(TOKT):
                do_tok(tt)

            names = {
                "x": x_d.tensor.name,
                "w": w_d.tensor.name,
                "b": b_d.tensor.name,
                "o": o_d.tensor.name,
            }

    nc.compile()
    return nc, names


def _get_built(key=(TOK_PC, OUT_PC, K)):
    if key not in _BUILD_CACHE:
        _BUILD_CACHE[key] = _build_bass(*key)
    return _BUILD_CACHE[key]


def make_in_maps(x, quantized_weight, bias, names):
    xf = np.ascontiguousarray(np.asarray(x, dtype=np.float32).reshape(B * S, K))
    w = np.ascontiguousarray(np.asarray(quantized_weight, dtype=np.int32))
    bs = np.asarray(bias, dtype=np.float32)
    in_maps = []
    for c in range(N_CORES):
        tg, og = divmod(c, OUT_GROUPS)
        in_maps.append({
            names["x"]: xf[tg * TOK_PC:(tg + 1) * TOK_PC],
            names["w"]: np.ascontiguousarray(w[og * OUT_PC:(og + 1) * OUT_PC]),
            names["b"]: np.ascontiguousarray(
                bs[og * OUT_PC:(og + 1) * OUT_PC].reshape(1, OUT_PC)),
        })
    return in_maps


def assemble_out(results, names):
    out = np.empty((B * S, O), np.float32)
    for c, r in enumerate(results):
        tg, og = divmod(c, OUT_GROUPS)
        out[tg * TOK_PC:(tg + 1) * TOK_PC, og * OUT_PC:(og + 1) * OUT_PC] = \
            r[names["o"]]
    return out.reshape(B, S, O)


def kernel(x, quantized_weight, bias):
    from concourse.bass_utils import run_bass_kernel_spmd

    nc, names = _get_built()
    in_maps = make_in_maps(x, quantized_weight, bias, names)
    res = run_bass_kernel_spmd(nc, in_maps, core_ids=list(range(N_CORES)))
    return assemble_out(res.results, names)


# revision 2
# speedup vs baseline: 1.0613x; 1.0613x over previous
"""Trainium2 Bass kernel for CustomQuantizedLinear.

Computes out[b,s,o] = sum_i x[b,s,i] * ((q[o,i]-128)*0.02) + bias[o]
for x (4,2048,4096) f32, q (4096,4096) int32, bias (4096,) f32.

Sharding across 8 NeuronCores: 4 token groups x 2 out-feature groups.
Each core computes a (2048 tokens, 2048 out-features) block of the
flattened (8192, 4096) output.

Per-core dataflow (matmul compute in bf16, accumulate f32):
  - weights: DMA int32 -> dequant to bf16 on ScalarE (one activation op:
    Copy(q*0.02 - 2.56)) -> PE-transpose 128x128 blocks (4 per PSUM
    chunk) -> single chunked copy to resident [K, out] bf16 tiles.
  - x: DMA f32 -> cast to bf16 (ScalarE) -> PE-transpose -> [K, tok]
    bf16 stationary tiles.
  - matmul: ki-outer / oc-inner so one stationary load feeds 4 N=512
    matmuls accumulating into 4 PSUM banks.
  - weight prep is interleaved with token 0's per-oc matmul groups so
    the PE never sits in a separate prep phase.
  - eviction: VectorE adds broadcast bias while copying PSUM->SBUF,
    then DMA to DRAM.
"""

import numpy as np

SCALE = 0.02
ZERO_POINT = 128

B, S, K, O = 4, 2048, 4096, 4096
N_CORES = 8
TOK_GROUPS, OUT_GROUPS = 4, 2
TOK_PC = B * S // TOK_GROUPS  # 2048 tokens per core
OUT_PC = O // OUT_GROUPS      # 2048 out features per core

_BUILD_CACHE = {}


def _build_bass(tok_pc=TOK_PC, out_pc=OUT_PC, k=K):
    """Build + compile the per-core Bass program. Returns (nc, names)."""
    from contextlib import ExitStack

    import concourse.mybir as mybir
    import concourse.tile as tile
    from concourse import bacc
    from concourse.masks import make_identity

    f32 = mybir.dt.float32
    bf16 = mybir.dt.bfloat16
    i32 = mybir.dt.int32
    ADD = mybir.AluOpType.add
    Copy = mybir.ActivationFunctionType.Copy

    P = 128
    FREE = 512                 # matmul moving free dim (one PSUM bank of f32)
    KT = k // P                # number of k tiles
    TOKT = tok_pc // P         # number of token tiles
    OC = out_pc // FREE        # out chunks of 512
    OT_PER_OC = FREE // P      # w row tiles per out chunk
    CH = 4                     # transposes per PSUM chunk copy
    HALF = min(k, 2048)        # staging width per DMA
    NH = k // HALF
    KL = HALF // P

    nc = bacc.Bacc(None, target_bir_lowering=False)
    with tile.TileContext(nc) as tc:
        with ExitStack() as ctx:
            dram = ctx.enter_context(tc.tile_pool(name="dram", bufs=1, space="DRAM"))
            x_d = dram.tile([tok_pc, k], f32, kind="ExternalInput", name="x_in")
            w_d = dram.tile([out_pc, k], i32, kind="ExternalInput", name="w_in")
            b_d = dram.tile([1, out_pc], f32, kind="ExternalInput", name="b_in")
            o_d = dram.tile([tok_pc, out_pc], f32, kind="ExternalOutput", name="o_out")

            const = ctx.enter_context(tc.tile_pool(name="const", bufs=1))
            stage = ctx.enter_context(tc.tile_pool(name="stage", bufs=3))
            bfst = ctx.enter_context(tc.tile_pool(name="bfst", bufs=3))
            wtp = ctx.enter_context(tc.tile_pool(name="wtp", bufs=1))
            xtp = ctx.enter_context(tc.tile_pool(name="xtp", bufs=2))
            outp = ctx.enter_context(tc.tile_pool(name="outp", bufs=4))
            pst = ctx.enter_context(tc.tile_pool(name="pst", bufs=2, space="PSUM"))
            psm = ctx.enter_context(tc.tile_pool(name="psm", bufs=6, space="PSUM"))

            ident = const.tile([P, P], bf16, name="ident")
            make_identity(nc, ident)
            ones_k = const.tile([1, P], f32, name="ones_k")
            nc.gpsimd.memset(ones_k, 1.0)

            # bias: load [1, out_pc], broadcast to 128 partitions via a
            # K=1 matmul with a ones stationary.
            bias_sb = const.tile([1, out_pc], f32, name="bias_sb")
            nc.sync.dma_start(bias_sb, b_d[:, :])
            bias_rep = const.tile([P, out_pc], f32, name="bias_rep")
            for oc in range(OC):
                pb = psm.tile([P, FREE], f32, tag="acc", name=f"pb{oc}")
                nc.tensor.matmul(
                    pb, lhsT=ones_k, rhs=bias_sb[:, oc * FREE:(oc + 1) * FREE],
                    start=True, stop=True,
                )
                nc.scalar.copy(bias_rep[:, oc * FREE:(oc + 1) * FREE], pb)

            # resident transposed dequantized weights: per out-chunk
            # [128 k-part, KT, 512 o] bf16
            wt = [wtp.tile([P, KT, FREE], bf16, name=f"wt{oc}") for oc in range(OC)]

            copy_flip = [0]

            def chunk_copy(dst, src):
                # alternate chunked PSUM->SBUF copies between DVE and ACT
                if copy_flip[0] % 2 == 0:
                    nc.vector.tensor_copy(dst, src)
                else:
                    nc.scalar.copy(dst, src)
                copy_flip[0] += 1

            def prep_w(ot):
                """Dequantize + transpose one 128-row slice of w into wt."""
                oc, col = divmod(ot, OT_PER_OC)
                for h in range(NH):
                    wstage = stage.tile([P, HALF], i32, tag="stage",
                                        name=f"wst_{ot}_{h}")
                    nc.sync.dma_start(
                        wstage, w_d[ot * P:(ot + 1) * P, h * HALF:(h + 1) * HALF])
                    wbf = bfst.tile([P, HALF], bf16, tag="bfst", name=f"wbf_{ot}_{h}")
                    nc.scalar.activation(
                        wbf, wstage, Copy,
                        bias=float(-ZERO_POINT * SCALE), scale=float(SCALE))
                    for kc in range(KL // CH):
                        pt = pst.tile([P, CH, P], bf16, tag="pt",
                                      name=f"wpt_{ot}_{h}_{kc}")
                        for j in range(CH):
                            kl = kc * CH + j
                            nc.tensor.transpose(
                                pt[:, j, :], wbf[:, kl * P:(kl + 1) * P], ident)
                        ki0 = h * KL + kc * CH
                        chunk_copy(
                            wt[oc][:, ki0:ki0 + CH, col * P:(col + 1) * P], pt)

            def make_xt(tt):
                """Load token tile tt, cast to bf16, transpose into [K, tok]."""
                xt = xtp.tile([P, KT, P], bf16, tag="xt", name=f"xt{tt}")
                for h in range(NH):
                    xstage = stage.tile([P, HALF], f32, tag="stage",
                                        name=f"xst_{tt}_{h}")
                    nc.sync.dma_start(
                        xstage, x_d[tt * P:(tt + 1) * P, h * HALF:(h + 1) * HALF])
                    xbf = bfst.tile([P, HALF], bf16, tag="bfst", name=f"xbf_{tt}_{h}")
                    nc.scalar.activation(xbf, xstage, Copy)
                    for kc in range(KL // CH):
                        pt = pst.tile([P, CH, P], bf16, tag="pt",
                                      name=f"xpt_{tt}_{h}_{kc}")
                        for j in range(CH):
                            kl = kc * CH + j
                            nc.tensor.transpose(
                                pt[:, j, :], xbf[:, kl * P:(kl + 1) * P], ident)
                        ki0 = h * KL + kc * CH
                        chunk_copy(xt[:, ki0:ki0 + CH, :], pt)
                return xt

            def evict(tt, oc, acc):
                ot_sb = outp.tile([P, FREE], f32, tag="outt", name=f"o_{tt}_{oc}")
                nc.vector.tensor_tensor(
                    ot_sb, acc, bias_rep[:, oc * FREE:(oc + 1) * FREE], ADD)
                nc.sync.dma_start(
                    o_d[tt * P:(tt + 1) * P, oc * FREE:(oc + 1) * FREE], ot_sb)

            # token 0: interleave per-oc weight prep with its matmul groups
            xt0 = make_xt(0)
            for oc in range(OC):
                for j in range(OT_PER_OC):
                    prep_w(oc * OT_PER_OC + j)
                acc = psm.tile([P, FREE], f32, tag="acc", name=f"acc_0_{oc}")
                for ki in range(KT):
                    nc.tensor.matmul(
                        acc, lhsT=xt0[:, ki, :], rhs=wt[oc][:, ki, :],
                        start=(ki == 0), stop=(ki == KT - 1))
                evict(0, oc, acc)

            # remaining tokens: ki-outer / oc-inner so each stationary
            # (xt[:, ki, :]) is reused by OC consecutive matmuls.
            for tt in range(1, TOKT):
                xt = make_xt(tt)
                accs = [psm.tile([P, FREE], f32, tag="acc", name=f"acc_{tt}_{oc}")
                        for oc in range(OC)]
                for ki in range(KT):
                    for oc in range(OC):
                        nc.tensor.matmul(
                            accs[oc], lhsT=xt[:, ki, :], rhs=wt[oc][:, ki, :],
                            start=(ki == 0), stop=(ki == KT - 1))
                for oc in range(OC):
                    evict(tt, oc, accs[oc])

            names = {
                "x": x_d.tensor.name,
                "w": w_d.tensor.name,
                "b": b_d.tensor.name,
                "o": o_d.tensor.name,
            }

    nc.compile()
    return nc, names


def _get_built(key=(TOK_PC, OUT_PC, K)):
    if key not in _BUILD_CACHE:
        _BUILD_CACHE[key] = _build_bass(*key)
    return _BUILD_CACHE[key]


def make_in_maps(x, quantized_weight, bias, names):
    xf = np.ascontiguousarray(np.asarray(x, dtype=np.float32).reshape(B * S, K))
    w = np.ascontiguousarray(np.asarray(quantized_weight, dtype=np.int32))
    bs = np.asarray(bias, dtype=np.float32)
    in_maps = []
    for c in range(N_CORES):
        tg, og = divmod(c, OUT_GROUPS)
        in_maps.append({
            names["x"]: xf[tg * TOK_PC:(tg + 1) * TOK_PC],
            names["w"]: np.ascontiguousarray(w[og * OUT_PC:(og + 1) * OUT_PC]),
            names["b"]: np.ascontiguousarray(
                bs[og * OUT_PC:(og + 1) * OUT_PC].reshape(1, OUT_PC)),
        })
    return in_maps


def assemble_out(results, names):
    out = np.empty((B * S, O), np.float32)
    for c, r in enumerate(results):
        tg, og = divmod(c, OUT_GROUPS)
        out[tg * TOK_PC:(tg + 1) * TOK_PC, og * OUT_PC:(og + 1) * OUT_PC] = \
            r[names["o"]]
    return out.reshape(B, S, O)


def kernel(x, quantized_weight, bias):
    from concourse.bass_utils import run_bass_kernel_spmd

    nc, names = _get_built()
    in_maps = make_in_maps(x, quantized_weight, bias, names)
    res = run_bass_kernel_spmd(nc, in_maps, core_ids=list(range(N_CORES)))
    return assemble_out(res.results, names)
